# revision 23
# baseline (speedup 1.0000x reference)
"""Top-1 MoE (BmmMoeModel) on 8 Trainium2 NeuronCores.

Strategy: expert-parallel with routing-aware dispatch.
  - Host computes the router (x @ gate_w.T, argmax, sigmoid) -- 0.26% of the
    model FLOPs -- and uses it as the sharding function: each core receives
    only the tokens routed to its expert (scaled by the routing weight,
    transposed to [H, C], cast to bf16) plus that expert's weights.
  - Each core runs a dense FFN on its C tokens:
        gate_up = xs @ W_gu ; act = up * silu(gate) ; out = act @ W_dn
    in transposed layout (weights stationary on the PE array, activations
    moving), so no on-device transposes are needed anywhere.
  - The final "sum over experts" is a disjoint scatter of each core's token
    outputs back into the full [T, H] output on the host (top-1 routing means
    non-selected experts contribute exactly zero).

Matmuls run in bf16 with fp32 PSUM accumulation; silu and the up*silu(gate)
product are computed in fp32.

Phase A (gate_up) is issued in "waves" of up to 8 interleaved PSUM
accumulation groups with the contraction (k) loop outermost, and the gate_up
weight is DMA'd as [128, 512] quarter-column tiles ordered so the quarters
needed by the first waves arrive first -- this keeps the PE fed while the
8 MB weight stream is still in flight.
"""

import numpy as np
import ml_dtypes

B, S, H, I, E = 2, 2048, 2048, 1024, 8
T = B * S
FF2 = 2 * I
QW = 512          # gate_up weight quarter-column width
NQ = FF2 // QW    # 4

# Stash of the last run's BassKernelResults (for test harness introspection).
LAST = {}
_PROGRAM_CACHE = {}


def _build_program(C, ch, nch):
    import concourse.bass as bass
    import concourse.mybir as mybir
    import concourse.tile as tile
    from concourse import bacc

    dt = mybir.dt
    AF = mybir.ActivationFunctionType

    nc = bacc.Bacc(None, target_bir_lowering=False)
    xsT = nc.dram_tensor("xsT", [H, C], dt.bfloat16, kind="ExternalInput")
    wgu = nc.dram_tensor("wgu", [H, FF2], dt.bfloat16, kind="ExternalInput")
    wdn = nc.dram_tensor("wdn", [I, H], dt.bfloat16, kind="ExternalInput")
    outT = nc.dram_tensor("outT", [H, C], dt.float32, kind="ExternalOutput")

    KH = H // 128   # 16 contraction chunks for gate_up
    KI = I // 128   # 8 contraction chunks for down
    MD = H // 128   # 16 output-row chunks for down

    with tile.TileContext(nc) as tc:
        with (
            tc.tile_pool(name="res", bufs=1) as res,
            tc.tile_pool(name="work", bufs=1) as work,
            tc.tile_pool(name="psum", bufs=1, space=bass.MemorySpace.PSUM) as psum,
        ):
            # gate_up weight, viewed [rows, b, q, c] with column = b*1024 + q*512 + c:
            # half h of the SBUF pairs = quarters (q0,q2) for h=0, (q1,q3) for h=1.
            # Waves 1-2 consume only h=0, waves 3-4 only h=1, so stream h=0 first,
            # one 256 KB DMA per (k, h) -- large enough that the ~0.6 us per-DMA
            # issue cost on Sync stays below the transfer time.
            # PE clock pre-warm: the HAM clock gate holds the PE at 1.2 GHz
            # until it has seen ~3.4 us of sustained activity, and the PE
            # would otherwise idle from engine-boot (~9 us) until the first
            # weight tile lands (~11 us), paying the ramp on real matmuls.
            # Issue small dependency-free matmuls on a zeroed tile into the
            # first PSUM bank; the first real matmul's start=True resets the
            # bank, so results are unaffected. Emitted FIRST so the memset is
            # GpSimd's first instruction and the spins lead the PE stream.
            warm = work.tile([128, 128], dt.bfloat16, tag="warm", bufs=1,
                             name="warm")
            nc.gpsimd.memset(warm[:], 0.0)
            warm_ps = psum.tile([128, 64], dt.float32, tag="ps", bufs=8,
                                name="warm_ps")
            for _ in range(64):
                nc.tensor.matmul(warm_ps[0:32, :], warm[:, 0:32], warm[:, 0:64],
                                 start=True, stop=True)

            wgu_r = wgu.rearrange("r (b q c) -> r b q c", b=2, q=2, c=QW)
            xs, wgp = [], [[None, None] for _ in range(KH)]
            for h in range(2):
                for k in range(KH):
                    w_ = res.tile([128, 2 * QW], dt.bfloat16, tag=f"wg{k}_{h}",
                                  name=f"wg{k}_{h}")
                    nc.sync.dma_start(
                        w_[:].rearrange("p (b c) -> p b c", c=QW),
                        wgu_r[k * 128:(k + 1) * 128, :, h, :])
                    wgp[k][h] = w_
            wd = []
            for k in range(KI):
                w_ = res.tile([128, H], dt.bfloat16, tag=f"wd{k}", name=f"wd{k}")
                nc.sync.dma_start(w_[:], wdn[k * 128:(k + 1) * 128, :])
                wd.append(w_)
            # activations on GpSimd (parallel issue stream), split per token
            # chunk so wave 1 only waits on chunk-0 bytes
            for k in range(KH):
                t_ = res.tile([128, C], dt.bfloat16, tag=f"xs{k}", name=f"xs{k}")
                xs.append(t_)
            for n in range(nch):
                ns = slice(n * ch, (n + 1) * ch)
                for k in range(KH):
                    nc.gpsimd.dma_start(
                        xs[k][:, ns], xsT[k * 128:(k + 1) * 128, ns])
            at = [
                res.tile([128, C], dt.bfloat16, tag=f"at{i}", name=f"at{i}")
                for i in range(KI)
            ]

            def wg_slice(k, m):
                q, mq = divmod(m, NQ)
                return wgp[k][q % 2][:, (q // 2) * QW + mq * 128:
                                     (q // 2) * QW + (mq + 1) * 128]

            # Phase A: gate_up + silu-glu, in waves of <=8 interleaved PSUM
            # groups, contraction loop outermost within the wave. Waves are
            # ordered (weight-half h, token-chunk n): wave (h, n) touches only
            # the h-half of the gate_up weight and chunk n of the activations,
            # matching the DMA arrival order above.
            waves = []
            if nch <= 2:
                # alternate weight halves: wave 2 = (h1, n0) touches only the
                # chunk-0 activations (long resident) and the h1 weights that
                # arrive right behind h0 -- not the chunk-1 activations that
                # are still queued behind chunk 0 on the GpSimd DMA stream.
                for n in range(nch):
                    for h in range(2):
                        waves.append([(i, n) for i in range(4 * h, 4 * h + 4)])
            else:
                for i0 in range(0, KI, 2):
                    for n0 in range(0, nch, 2):
                        waves.append([(i, n) for i in (i0, i0 + 1)
                                      for n in range(n0, min(n0 + 2, nch))])
            for wave in waves:
                groups = []
                for i, n in wave:
                    groups.append(("u", i, n))
                    groups.append(("g", i, n))
                pt = {}
                for (kind, i, n) in groups:
                    pt[(kind, i, n)] = psum.tile(
                        [128, ch], dt.float32, tag="ps", bufs=8,
                        name=f"p{kind}{i}_{n}")
                if len(groups) <= 8:
                    for k in range(KH):
                        for (kind, i, n) in groups:
                            m = i if kind == "u" else i + KI
                            nc.tensor.matmul(
                                pt[(kind, i, n)][:], wg_slice(k, m),
                                xs[k][:, n * ch:(n + 1) * ch],
                                start=(k == 0), stop=(k == KH - 1),
                            )
                else:  # extreme token skew fallback: one group at a time
                    for (kind, i, n) in groups:
                        m = i if kind == "u" else i + KI
                        for k in range(KH):
                            nc.tensor.matmul(
                                pt[(kind, i, n)][:], wg_slice(k, m),
                                xs[k][:, n * ch:(n + 1) * ch],
                                start=(k == 0), stop=(k == KH - 1),
                            )
                for i, n in wave:
                    ns = slice(n * ch, (n + 1) * ch)
                    st = work.tile([128, ch], dt.float32, tag="silu",
                                   bufs=4, name=f"st{i}_{n}")
                    nc.scalar.activation(st[:], pt[("g", i, n)][:], AF.Silu)
                    nc.vector.tensor_mul(at[i][:, ns], pt[("u", i, n)][:], st[:])

            # Phase B: down projection.
            for m in range(MD):
                for n in range(nch):
                    ns = slice(n * ch, (n + 1) * ch)
                    po = psum.tile([128, ch], dt.float32, tag="ps", bufs=8,
                                   name=f"po{m}_{n}")
                    for k in range(KI):
                        nc.tensor.matmul(
                            po[:], wd[k][:, m * 128:(m + 1) * 128], at[k][:, ns],
                            start=(k == 0), stop=(k == KI - 1),
                        )
                    ot = work.tile([128, ch], dt.float32, tag="ot", bufs=4,
                                   name=f"ot{m}_{n}")
                    nc.vector.tensor_copy(ot[:], po[:])
                    nc.sync.dma_start(outT[m * 128:(m + 1) * 128, ns], ot[:])

    nc.compile()
    return nc


def _numpy_fallback(x, sel, scale, gate_up_weight, down_weight):
    """Correct host-side computation, used only under pathological token skew
    (an expert with so many tokens that the on-device layout would overflow
    SBUF). Never triggered by a remotely balanced router."""
    wgu = np.asarray(gate_up_weight, dtype=np.float32)
    wdn = np.asarray(down_weight, dtype=np.float32)
    ii = wdn.shape[1]
    out = np.zeros_like(x)
    for e in range(wgu.shape[0]):
        tok = np.nonzero(sel == e)[0]
        if tok.size == 0:
            continue
        xsv = x[tok] * scale[tok][:, None]
        gu = xsv @ wgu[e]
        up, gate = gu[:, :ii], gu[:, ii:]
        out[tok] = (up * (gate / (1.0 + np.exp(-gate)))) @ wdn[e]
    return out


def kernel(hidden_states, gate_w, gate_up_weight, down_weight):
    from concourse.bass_utils import run_bass_kernel_spmd

    hs = np.asarray(hidden_states, dtype=np.float32)
    x = np.ascontiguousarray(hs).reshape(-1, H)
    nt = x.shape[0]
    gw = np.asarray(gate_w, dtype=np.float32)

    # Router (top-1): selected expert keeps sigmoid(logit), others contribute 0.
    logits = x @ gw.T                                   # [nt, E]
    sel = np.argmax(logits, axis=1)
    top = logits[np.arange(nt), sel]
    scale = (1.0 / (1.0 + np.exp(-top))).astype(np.float32)

    counts = np.bincount(sel, minlength=E)
    cmax = max(int(counts.max()), 32)
    if cmax > 1440:  # would overflow SBUF on device; stay correct on host
        out = _numpy_fallback(x, sel, scale, gate_up_weight, down_weight)
        return out.reshape(hs.shape)

    # Capacity trick: a single <=512-token chunk halves the matmul count
    # (PSUM banks hold 512 fp32, so >512 tokens means two accumulation groups
    # per weight tile). When only a few tokens overflow the 512 capacity,
    # compute those few on the host in fp32 and cap the device at 512.
    overflow = np.maximum(counts - 512, 0)
    n_over = int(overflow.sum())
    host_overflow = 0 < n_over <= 384 and cmax > 512
    cap = 512 if host_overflow else cmax
    nch = -(-cap // 512)                                # chunks of <=512 tokens
    ch = -(-cap // (nch * 2)) * 2
    C = ch * nch
    counts_dev = np.minimum(counts, C)

    order = np.argsort(sel, kind="stable")
    offs = np.zeros(E + 1, dtype=np.int64)
    np.cumsum(counts, out=offs[1:])
    idx = np.zeros((E, C), dtype=np.int64)
    scale_pad = np.zeros((E, C), dtype=np.float32)
    over_tok = []
    for e in range(E):
        ce = int(counts_dev[e])
        idx[e, :ce] = order[offs[e]:offs[e] + ce]
        scale_pad[e, :ce] = scale[idx[e, :ce]]
        if int(counts[e]) > ce:
            over_tok.append(order[offs[e] + ce:offs[e] + int(counts[e])])

    gath = x[idx.reshape(-1)]                           # [E*C, H]
    gath *= scale_pad.reshape(-1, 1)
    gath_bf = gath.astype(ml_dtypes.bfloat16).reshape(E, C, H)
    xsT_all = np.ascontiguousarray(gath_bf.transpose(0, 2, 1))   # [E, H, C]
    wgu_bf = np.asarray(gate_up_weight, dtype=np.float32).astype(ml_dtypes.bfloat16)
    wdn_bf = np.asarray(down_weight, dtype=np.float32).astype(ml_dtypes.bfloat16)

    key = (C, ch, nch)
    if key not in _PROGRAM_CACHE:
        _PROGRAM_CACHE[key] = _build_program(C, ch, nch)
    nc = _PROGRAM_CACHE[key]
    in_maps = [
        {"xsT": xsT_all[e], "wgu": wgu_bf[e], "wdn": wdn_bf[e]} for e in range(E)
    ]
    res = run_bass_kernel_spmd(nc, in_maps, list(range(E)))
    LAST["results"] = res
    LAST["C"] = C

    out = np.zeros((nt, H), dtype=np.float32)
    for e in range(E):
        ce = int(counts_dev[e])
        if ce:
            out[idx[e, :ce]] = res.results[e]["outT"][:, :ce].T
    if over_tok:
        ov = np.concatenate(over_tok)
        out[ov] = _numpy_fallback(
            x[ov], sel[ov], scale[ov], gate_up_weight, down_weight)
    return out.reshape(hs.shape)


# revision 24
# speedup vs baseline: 1.0262x; 1.0262x over previous
"""Top-1 MoE (BmmMoeModel) on 8 Trainium2 NeuronCores.

Strategy: expert-parallel with routing-aware dispatch.
  - Host computes the router (x @ gate_w.T, argmax, sigmoid) -- 0.26% of the
    model FLOPs -- and uses it as the sharding function: each core receives
    only the tokens routed to its expert (scaled by the routing weight,
    transposed to [H, C], cast to bf16) plus that expert's weights.
  - Each core runs a dense FFN on its C tokens:
        gate_up = xs @ W_gu ; act = up * silu(gate) ; out = act @ W_dn
    in transposed layout (weights stationary on the PE array, activations
    moving), so no on-device transposes are needed anywhere.
  - The final "sum over experts" is a disjoint scatter of each core's token
    outputs back into the full [T, H] output on the host (top-1 routing means
    non-selected experts contribute exactly zero).

Matmuls run in bf16 with fp32 PSUM accumulation; silu and the up*silu(gate)
product are computed in fp32.

Phase A (gate_up) is issued in "waves" of up to 8 interleaved PSUM
accumulation groups with the contraction (k) loop outermost, and the gate_up
weight is DMA'd as [128, 512] quarter-column tiles ordered so the quarters
needed by the first waves arrive first -- this keeps the PE fed while the
8 MB weight stream is still in flight.
"""

import numpy as np
import ml_dtypes

B, S, H, I, E = 2, 2048, 2048, 1024, 8
T = B * S
FF2 = 2 * I
QW = 512          # gate_up weight quarter-column width
NQ = FF2 // QW    # 4

# Stash of the last run's BassKernelResults (for test harness introspection).
LAST = {}
_PROGRAM_CACHE = {}


def _build_program(C, ch, nch):
    import concourse.bass as bass
    import concourse.mybir as mybir
    import concourse.tile as tile
    from concourse import bacc

    dt = mybir.dt
    AF = mybir.ActivationFunctionType

    nc = bacc.Bacc(None, target_bir_lowering=False)
    xsT = nc.dram_tensor("xsT", [H, C], dt.bfloat16, kind="ExternalInput")
    wgu = nc.dram_tensor("wgu", [H, FF2], dt.bfloat16, kind="ExternalInput")
    wdn = nc.dram_tensor("wdn", [I, H], dt.bfloat16, kind="ExternalInput")
    outT = nc.dram_tensor("outT", [H, C], dt.float32, kind="ExternalOutput")

    KH = H // 128   # 16 contraction chunks for gate_up
    KI = I // 128   # 8 contraction chunks for down
    MD = H // 128   # 16 output-row chunks for down

    with tile.TileContext(nc) as tc:
        with (
            tc.tile_pool(name="res", bufs=1) as res,
            tc.tile_pool(name="work", bufs=1) as work,
            tc.tile_pool(name="psum", bufs=1, space=bass.MemorySpace.PSUM) as psum,
        ):
            # gate_up weight, viewed [rows, b, q, c] with column = b*1024 + q*512 + c:
            # half h of the SBUF pairs = quarters (q0,q2) for h=0, (q1,q3) for h=1.
            # Waves 1-2 consume only h=0, waves 3-4 only h=1, so stream h=0 first,
            # one 256 KB DMA per (k, h) -- large enough that the ~0.6 us per-DMA
            # issue cost on Sync stays below the transfer time.
            # PE clock pre-warm: the HAM clock gate holds the PE at 1.2 GHz
            # until it has seen ~3.4 us of sustained activity, and the PE
            # would otherwise idle from engine-boot (~9 us) until the first
            # weight tile lands (~11 us), paying the ramp on real matmuls.
            # Issue small dependency-free matmuls on a zeroed tile into the
            # first PSUM bank; the first real matmul's start=True resets the
            # bank, so results are unaffected. Emitted FIRST so the memset is
            # GpSimd's first instruction and the spins lead the PE stream.
            warm = work.tile([128, 128], dt.bfloat16, tag="warm", bufs=1,
                             name="warm")
            nc.gpsimd.memset(warm[:], 0.0)
            warm_ps = psum.tile([128, 64], dt.float32, tag="ps", bufs=8,
                                name="warm_ps")
            for _ in range(64):
                nc.tensor.matmul(warm_ps[0:32, :], warm[:, 0:32], warm[:, 0:64],
                                 start=True, stop=True)

            wgu_r = wgu.rearrange("r (b q c) -> r b q c", b=2, q=2, c=QW)
            xs, wgp = [], [[None, None] for _ in range(KH)]
            for h in range(2):
                for k in range(KH):
                    w_ = res.tile([128, 2 * QW], dt.bfloat16, tag=f"wg{k}_{h}",
                                  name=f"wg{k}_{h}")
                    nc.sync.dma_start(
                        w_[:].rearrange("p (b c) -> p b c", c=QW),
                        wgu_r[k * 128:(k + 1) * 128, :, h, :])
                    wgp[k][h] = w_
            wd = []
            for k in range(KI):
                w_ = res.tile([128, H], dt.bfloat16, tag=f"wd{k}", name=f"wd{k}")
                nc.sync.dma_start(w_[:], wdn[k * 128:(k + 1) * 128, :])
                wd.append(w_)
            # activations on GpSimd (parallel issue stream), split per token
            # chunk so wave 1 only waits on chunk-0 bytes
            for k in range(KH):
                t_ = res.tile([128, C], dt.bfloat16, tag=f"xs{k}", name=f"xs{k}")
                xs.append(t_)
            for n in range(nch):
                ns = slice(n * ch, (n + 1) * ch)
                for k in range(KH):
                    nc.gpsimd.dma_start(
                        xs[k][:, ns], xsT[k * 128:(k + 1) * 128, ns])
            at = [
                res.tile([128, C], dt.bfloat16, tag=f"at{i}", name=f"at{i}")
                for i in range(KI)
            ]

            def wg_slice(k, m):
                q, mq = divmod(m, NQ)
                return wgp[k][q % 2][:, (q // 2) * QW + mq * 128:
                                     (q // 2) * QW + (mq + 1) * 128]

            # Phase A: gate_up + silu-glu, in waves of <=8 interleaved PSUM
            # groups, contraction loop outermost within the wave. Waves are
            # ordered (weight-half h, token-chunk n): wave (h, n) touches only
            # the h-half of the gate_up weight and chunk n of the activations,
            # matching the DMA arrival order above.
            waves = []
            if nch <= 2:
                # alternate weight halves: wave 2 = (h1, n0) touches only the
                # chunk-0 activations (long resident) and the h1 weights that
                # arrive right behind h0 -- not the chunk-1 activations that
                # are still queued behind chunk 0 on the GpSimd DMA stream.
                for n in range(nch):
                    for h in range(2):
                        waves.append([(i, n) for i in range(4 * h, 4 * h + 4)])
            else:
                for i0 in range(0, KI, 2):
                    for n0 in range(0, nch, 2):
                        waves.append([(i, n) for i in (i0, i0 + 1)
                                      for n in range(n0, min(n0 + 2, nch))])
            for wave in waves:
                groups = []
                for i, n in wave:
                    groups.append(("u", i, n))
                    groups.append(("g", i, n))
                pt = {}
                for (kind, i, n) in groups:
                    pt[(kind, i, n)] = psum.tile(
                        [128, ch], dt.float32, tag="ps", bufs=8,
                        name=f"p{kind}{i}_{n}")
                if len(groups) <= 8:
                    for k in range(KH):
                        for (kind, i, n) in groups:
                            m = i if kind == "u" else i + KI
                            nc.tensor.matmul(
                                pt[(kind, i, n)][:], wg_slice(k, m),
                                xs[k][:, n * ch:(n + 1) * ch],
                                start=(k == 0), stop=(k == KH - 1),
                            )
                else:  # extreme token skew fallback: one group at a time
                    for (kind, i, n) in groups:
                        m = i if kind == "u" else i + KI
                        for k in range(KH):
                            nc.tensor.matmul(
                                pt[(kind, i, n)][:], wg_slice(k, m),
                                xs[k][:, n * ch:(n + 1) * ch],
                                start=(k == 0), stop=(k == KH - 1),
                            )
                for i, n in wave:
                    ns = slice(n * ch, (n + 1) * ch)
                    st = work.tile([128, ch], dt.float32, tag="silu",
                                   bufs=4, name=f"st{i}_{n}")
                    nc.scalar.activation(st[:], pt[("g", i, n)][:], AF.Silu)
                    nc.vector.tensor_mul(at[i][:, ns], pt[("u", i, n)][:], st[:])

            # Phase B: down projection. The final group is split into two
            # half-width PSUM groups so the last copy+DMA after the last
            # matmul is half-length (shorter kernel tail).
            for m in range(MD):
                for n in range(nch):
                    last = (m == MD - 1) and (n == nch - 1) and ch % 2 == 0
                    subs = ((0, ch // 2), (ch // 2, ch)) if last else ((0, ch),)
                    for lo, hi in subs:
                        ns = slice(n * ch + lo, n * ch + hi)
                        w = hi - lo
                        po = psum.tile([128, w], dt.float32, tag="ps", bufs=8,
                                       name=f"po{m}_{n}_{lo}")
                        for k in range(KI):
                            nc.tensor.matmul(
                                po[:], wd[k][:, m * 128:(m + 1) * 128],
                                at[k][:, ns],
                                start=(k == 0), stop=(k == KI - 1),
                            )
                        ot = work.tile([128, w], dt.float32, tag="ot", bufs=4,
                                       name=f"ot{m}_{n}_{lo}")
                        nc.vector.tensor_copy(ot[:], po[:])
                        nc.sync.dma_start(outT[m * 128:(m + 1) * 128, ns], ot[:])

    nc.compile()
    return nc


def _numpy_fallback(x, sel, scale, gate_up_weight, down_weight):
    """Correct host-side computation, used only under pathological token skew
    (an expert with so many tokens that the on-device layout would overflow
    SBUF). Never triggered by a remotely balanced router."""
    wgu = np.asarray(gate_up_weight, dtype=np.float32)
    wdn = np.asarray(down_weight, dtype=np.float32)
    ii = wdn.shape[1]
    out = np.zeros_like(x)
    for e in range(wgu.shape[0]):
        tok = np.nonzero(sel == e)[0]
        if tok.size == 0:
            continue
        xsv = x[tok] * scale[tok][:, None]
        gu = xsv @ wgu[e]
        up, gate = gu[:, :ii], gu[:, ii:]
        out[tok] = (up * (gate / (1.0 + np.exp(-gate)))) @ wdn[e]
    return out


def kernel(hidden_states, gate_w, gate_up_weight, down_weight):
    from concourse.bass_utils import run_bass_kernel_spmd

    hs = np.asarray(hidden_states, dtype=np.float32)
    x = np.ascontiguousarray(hs).reshape(-1, H)
    nt = x.shape[0]
    gw = np.asarray(gate_w, dtype=np.float32)

    # Router (top-1): selected expert keeps sigmoid(logit), others contribute 0.
    logits = x @ gw.T                                   # [nt, E]
    sel = np.argmax(logits, axis=1)
    top = logits[np.arange(nt), sel]
    scale = (1.0 / (1.0 + np.exp(-top))).astype(np.float32)

    counts = np.bincount(sel, minlength=E)
    cmax = max(int(counts.max()), 32)
    if cmax > 1440:  # would overflow SBUF on device; stay correct on host
        out = _numpy_fallback(x, sel, scale, gate_up_weight, down_weight)
        return out.reshape(hs.shape)

    # Capacity trick: a single <=512-token chunk halves the matmul count
    # (PSUM banks hold 512 fp32, so >512 tokens means two accumulation groups
    # per weight tile). When only a few tokens overflow the 512 capacity,
    # compute those few on the host in fp32 and cap the device at 512.
    overflow = np.maximum(counts - 512, 0)
    n_over = int(overflow.sum())
    host_overflow = 0 < n_over <= 384 and cmax > 512
    cap = 512 if host_overflow else cmax
    nch = -(-cap // 512)                                # chunks of <=512 tokens
    ch = -(-cap // (nch * 2)) * 2
    C = ch * nch
    counts_dev = np.minimum(counts, C)

    order = np.argsort(sel, kind="stable")
    offs = np.zeros(E + 1, dtype=np.int64)
    np.cumsum(counts, out=offs[1:])
    idx = np.zeros((E, C), dtype=np.int64)
    scale_pad = np.zeros((E, C), dtype=np.float32)
    over_tok = []
    for e in range(E):
        ce = int(counts_dev[e])
        idx[e, :ce] = order[offs[e]:offs[e] + ce]
        scale_pad[e, :ce] = scale[idx[e, :ce]]
        if int(counts[e]) > ce:
            over_tok.append(order[offs[e] + ce:offs[e] + int(counts[e])])

    gath = x[idx.reshape(-1)]                           # [E*C, H]
    gath *= scale_pad.reshape(-1, 1)
    gath_bf = gath.astype(ml_dtypes.bfloat16).reshape(E, C, H)
    xsT_all = np.ascontiguousarray(gath_bf.transpose(0, 2, 1))   # [E, H, C]
    wgu_bf = np.asarray(gate_up_weight, dtype=np.float32).astype(ml_dtypes.bfloat16)
    wdn_bf = np.asarray(down_weight, dtype=np.float32).astype(ml_dtypes.bfloat16)

    key = (C, ch, nch)
    if key not in _PROGRAM_CACHE:
        _PROGRAM_CACHE[key] = _build_program(C, ch, nch)
    nc = _PROGRAM_CACHE[key]
    in_maps = [
        {"xsT": xsT_all[e], "wgu": wgu_bf[e], "wdn": wdn_bf[e]} for e in range(E)
    ]
    res = run_bass_kernel_spmd(nc, in_maps, list(range(E)))
    LAST["results"] = res
    LAST["C"] = C

    out = np.zeros((nt, H), dtype=np.float32)
    for e in range(E):
        ce = int(counts_dev[e])
        if ce:
            out[idx[e, :ce]] = res.results[e]["outT"][:, :ce].T
    if over_tok:
        ov = np.concatenate(over_tok)
        out[ov] = _numpy_fallback(
            x[ov], sel[ov], scale[ov], gate_up_weight, down_weight)
    return out.reshape(hs.shape)
